# revision 1
# baseline (speedup 1.0000x reference)
"""GNN message-passing (std aggregator) on 8 TRN2 NeuronCores.

Math per target node: count, S1 = sum x[src], S2 = sum x[src]^2;
mean = S1/max(count,eps); var = S2/count - mean^2;
std = sqrt(max(var,0)), zeroed where count <= 1.

Strategy: shard TARGET nodes across cores (no collectives). Host packs nodes
into 128-bin blocks balanced by in-degree (serpentine deal), buckets edges by
(block, src-quarter) with uniform tile capacity tq per (block,quarter) so one
NEFF serves all cores. Device per core, per group of GB blocks:
  - 4x dma_gather (one per src quarter of x; int16 idx < 25000) pulls
    x[src] rows into SBUF in quarter-major column order,
  - ACT builds rhs tiles [x | x^2 | 1] (cast to MM dtype),
  - DVE builds 4-wide one-hot tiles (slot-vs-iota is_equal),
  - PE matmul-accumulates [128 bins x 129] = [S1 | S2 | count] in PSUM,
  - small DVE/ACT finishing pass computes std, DMA out per block.
"""

import numpy as np

N_NODES = 100000
N_FEAT = 64
N_EDGES = 1600000
P = 128
NCORES = 8
NB = 98                 # blocks per core
NBLK = NCORES * NB      # 784
GB = 7                  # blocks per group; 98 = 14*7
NQUART = 4
NQ = N_NODES // NQUART  # rows per src quarter (25000 < 32768 for int16 idx)
EPS = 1e-8
MM_DT = "bfloat16"      # "float32" | "bfloat16" for matmul operands

_CACHE = {}


def _build_program(n_nodes, f, nb, tq, gb, nq, mm_dt):
    import concourse.bass as bass
    import concourse.bacc as bacc
    import concourse.mybir as mybir
    import concourse.tile as tile

    F32 = mybir.dt.float32
    I16 = mybir.dt.int16
    MDT = getattr(mybir.dt, mm_dt)
    AO = mybir.AluOpType
    AF = mybir.ActivationFunctionType

    t = NQUART * tq            # tiles (columns) per block
    W = 2 * f + 1              # 129
    C = nb * t                 # total columns per core
    gcols = gb * t             # columns per group
    qcols = gb * tq            # columns per (group, quarter)
    ng = nb // gb
    nidx = qcols * P           # indices per gather
    i16c = nidx // 16          # idx16 cols per gather

    nc = bacc.Bacc()
    xd = nc.declare_dram_parameter("x", [n_nodes, f], F32, isOutput=False)
    gidxd = nc.declare_dram_parameter(
        "gidx", [P, ng * NQUART * i16c], I16, isOutput=False)
    tgtd = nc.declare_dram_parameter("tgt", [P, C], F32, isOutput=False)
    outd = nc.declare_dram_parameter("out", [nb * P, f], F32, isOutput=True)

    with tile.TileContext(nc) as tc:
        with (
            tc.tile_pool(name="const", bufs=1) as constp,
            tc.tile_pool(name="io", bufs=2) as iop,
            tc.tile_pool(name="msg", bufs=2) as msgp,
            tc.tile_pool(name="oh", bufs=6) as ohp,
            tc.tile_pool(name="fin", bufs=4) as finp,
            tc.tile_pool(name="ov", bufs=4) as ovp,
            tc.tile_pool(name="ps", bufs=8, space="PSUM") as psump,
        ):
            # 4-wide iota [128, 4*128]: value = column index % 128
            iota4 = constp.tile([P, 4 * P], F32)
            nc.gpsimd.iota(iota4[:], pattern=[[0, 4], [1, P]], base=0,
                           channel_multiplier=0,
                           allow_small_or_imprecise_dtypes=True)

            for g in range(ng):
                idx = iop.tile([P, NQUART * i16c], I16, tag="idx")
                tg = iop.tile([P, gcols], F32, tag="tg")
                nc.sync.dma_start(
                    out=idx[:],
                    in_=gidxd[:, g * NQUART * i16c:(g + 1) * NQUART * i16c])
                nc.sync.dma_start(
                    out=tg[:], in_=tgtd[:, g * gcols:(g + 1) * gcols])
                tgv = iop.tile([P, gcols], F32, tag="tgv")
                nc.vector.tensor_copy(out=tgv[:], in_=tg[:])

                gbuf = msgp.tile([P, gcols * f], F32, tag="g")
                g3 = gbuf[:].rearrange("p (c e) -> p c e", e=f)
                for qq in range(NQUART):
                    nc.gpsimd.dma_gather(
                        out_ap=g3[:, qq * qcols:(qq + 1) * qcols, :],
                        in_ap=xd[qq * nq:(qq + 1) * nq, :],
                        idxs_ap=idx[:, qq * i16c:(qq + 1) * i16c],
                        num_idxs=nidx,
                        num_idxs_reg=nidx,
                        elem_size=f,
                        single_packet=False,
                    )
                sqx = msgp.tile([P, gcols * W], MDT, tag="sqx")
                s3 = sqx[:].rearrange("p (c w) -> p c w", w=W)
                nc.scalar.activation(out=s3[:, :, 0:f], in_=g3[:, :, :],
                                     func=AF.Copy)
                nc.scalar.square(out=s3[:, :, f:2 * f], in_=g3[:, :, :])
                nc.scalar.activation(out=s3[:, :, 2 * f:W], in_=g3[:, :, 0:1],
                                     func=AF.Copy, bias=1.0, scale=0.0)

                pss = [psump.tile([P, W], F32, tag="ps", name=f"ps_{g}_{bl}")
                       for bl in range(gb)]
                for pk in range(gcols // 4):
                    oh4 = ohp.tile([P, 4 * P], MDT)
                    nc.vector.tensor_tensor(
                        out=oh4[:].rearrange("p (c e) -> p c e", e=P),
                        in0=tgv[:, 4 * pk:4 * pk + 4]
                            .rearrange("p (c u) -> p c u", u=1)
                            .to_broadcast([P, 4, P]),
                        in1=iota4[:].rearrange("p (c e) -> p c e", e=P),
                        op=AO.is_equal,
                    )
                    for i in range(4):
                        cl = 4 * pk + i
                        qq = cl // qcols
                        r = cl % qcols
                        bl = r // tq
                        j = r % tq
                        nc.tensor.matmul(
                            out=pss[bl][:],
                            lhsT=oh4[:, i * P:(i + 1) * P],
                            rhs=sqx[:, cl * W:(cl + 1) * W],
                            start=(qq == 0 and j == 0),
                            stop=(qq == NQUART - 1 and j == tq - 1),
                        )
                for bl in range(gb):
                    b = g * gb + bl
                    ps = pss[bl]
                    cnt = finp.tile([P, 1], F32, tag="cnt")
                    nc.vector.tensor_scalar(
                        out=cnt[:], in0=ps[:, 2 * f:W],
                        scalar1=float(EPS), scalar2=None, op0=AO.max)
                    rec = finp.tile([P, 1], F32, tag="rec")
                    nc.vector.reciprocal(out=rec[:], in_=cnt[:])
                    mean = finp.tile([P, f], F32, tag="mean")
                    nc.vector.tensor_scalar_mul(
                        out=mean[:], in0=ps[:, 0:f], scalar1=rec[:])
                    ex2 = finp.tile([P, f], F32, tag="ex2")
                    nc.vector.tensor_scalar_mul(
                        out=ex2[:], in0=ps[:, f:2 * f], scalar1=rec[:])
                    var = finp.tile([P, f], F32, tag="var")
                    nc.vector.tensor_tensor(
                        out=var[:], in0=mean[:], in1=mean[:], op=AO.mult)
                    nc.vector.tensor_tensor(
                        out=var[:], in0=ex2[:], in1=var[:], op=AO.subtract)
                    nc.vector.tensor_scalar(
                        out=var[:], in0=var[:], scalar1=0.0, scalar2=None,
                        op0=AO.max)
                    std = ovp.tile([P, f], F32, tag="std")
                    nc.scalar.sqrt(out=std[:], in_=var[:])
                    mask = finp.tile([P, 1], F32, tag="mask")
                    nc.vector.tensor_scalar(
                        out=mask[:], in0=ps[:, 2 * f:W],
                        scalar1=1.5, scalar2=None, op0=AO.is_gt)
                    nc.vector.tensor_scalar_mul(
                        out=std[:], in0=std[:], scalar1=mask[:])
                    nc.sync.dma_start(
                        out=outd[b * P:(b + 1) * P, :], in_=std[:])
    return nc


def _host_prep(x, edge_index):
    src = np.asarray(edge_index[0], dtype=np.int64)
    tgt = np.asarray(edge_index[1], dtype=np.int64)
    n_edges = src.shape[0]
    counts = np.bincount(tgt, minlength=N_NODES)

    # serpentine deal of count-sorted nodes into NBLK blocks of <=128 slots
    order = np.argsort(-counts, kind="stable")
    ranks = np.arange(N_NODES)
    rounds = ranks // NBLK
    pos = ranks % NBLK
    blk_of_rank = np.where(rounds % 2 == 0, pos, NBLK - 1 - pos)
    blk = np.empty(N_NODES, np.int64)
    slot = np.empty(N_NODES, np.int64)
    blk[order] = blk_of_rank
    slot[order] = rounds
    assert slot.max() < P

    eb = blk[tgt]                      # edge -> block
    eq = src // NQ                     # edge -> src quarter
    es = slot[tgt]                     # edge -> slot in block
    seg = eb * NQUART + eq             # edge -> (block, quarter) segment
    segsums = np.bincount(seg, minlength=NBLK * NQUART)
    tq = int(np.ceil(segsums.max() / P))
    cap = tq * P

    order_e = np.argsort(seg, kind="stable")
    segs = seg[order_e]
    starts = np.zeros(NBLK * NQUART, np.int64)
    np.cumsum(segsums[:-1], out=starts[1:])
    within = np.arange(n_edges) - starts[segs]
    flat = segs * cap + within

    gidxq = np.zeros((NBLK, NQUART, cap), np.int16)
    tgtq = np.full((NBLK, NQUART, cap), -1.0, np.float32)
    gidxq.reshape(-1)[flat] = (src[order_e] % NQ).astype(np.int16)
    tgtq.reshape(-1)[flat] = es[order_e].astype(np.float32)

    xf = np.ascontiguousarray(np.asarray(x, dtype=np.float32))
    ng = NB // GB
    i16c = GB * cap // 16

    in_maps = []
    for c in range(NCORES):
        tb = tgtq[c * NB:(c + 1) * NB]          # [NB, 4, cap]
        gi = gidxq[c * NB:(c + 1) * NB]
        # tgt columns: (group, quarter, block, tile) -> [P, C]
        tcore = (tb.reshape(ng, GB, NQUART, cap)
                 .transpose(0, 2, 1, 3)          # [ng, 4, GB, cap]
                 .reshape(ng * NQUART * GB * tq, P).T)
        # idx16: per (group, quarter): stream of GB*cap idxs wrapped %16
        gs = (gi.reshape(ng, GB, NQUART, cap)
              .transpose(0, 2, 1, 3)             # [ng, 4, GB, cap]
              .reshape(ng * NQUART, GB * cap))   # per-gather streams
        idx16 = np.ascontiguousarray(
            np.tile(gs.reshape(ng * NQUART, i16c, 16).transpose(0, 2, 1)
                    .reshape(ng * NQUART * 16, i16c)
                    .reshape(ng * NQUART, 16, i16c)
                    .transpose(1, 0, 2).reshape(16, ng * NQUART * i16c),
                    (8, 1)))
        in_maps.append({
            "x": xf,
            "gidx": idx16,
            "tgt": np.ascontiguousarray(tcore),
        })
    return tq, in_maps, blk, slot


def _run(x, edge_index, trace=False):
    from concourse.bass_utils import run_bass_kernel_spmd

    tq, in_maps, blk, slot = _host_prep(x, edge_index)
    key = ("prog", tq, MM_DT)
    if key not in _CACHE:
        nc_ = _build_program(N_NODES, N_FEAT, NB, tq, GB, NQ, MM_DT)
        nc_.finalize()
        _CACHE[key] = nc_
    nc = _CACHE[key]
    res = run_bass_kernel_spmd(
        nc, in_maps, core_ids=list(range(NCORES)), trace=trace)

    outs = [np.asarray(r["out"]) for r in res.results]
    out_full = np.empty((N_NODES, N_FEAT), np.float32)
    cores = blk // NB
    rows = (blk % NB) * P + slot
    for c in range(NCORES):
        m = cores == c
        out_full[m] = outs[c][rows[m]]
    return out_full, res


def kernel(**inputs):
    out, _ = _run(inputs["x"], inputs["edge_index"], trace=False)
    return out



# revision 3
# speedup vs baseline: 2.3332x; 2.3332x over previous
"""GNN message-passing (std aggregator) on 8 TRN2 NeuronCores.

Math per target node: count, S1 = sum x[src], S2 = sum x[src]^2;
mean = S1/max(count,eps); var = S2/count - mean^2;
std = sqrt(max(var,0)), zeroed where count <= 1.

Strategy: shard TARGET nodes across cores (no collectives). Host packs nodes
into 128-bin blocks balanced by in-degree (serpentine deal), buckets edges by
(block, src-quarter) with uniform tile capacity tq per (block,quarter) so one
NEFF serves all cores. Device per core, per group of GB blocks:
  - 4x dma_gather (one per src quarter of x; int16 idx < 25000) pulls
    x[src] rows into SBUF in quarter-major column order,
  - ACT builds rhs tiles [x | x^2 | 1] (cast to MM dtype),
  - DVE builds 4-wide one-hot tiles (slot-vs-iota is_equal),
  - PE matmul-accumulates [128 bins x 129] = [S1 | S2 | count] in PSUM,
  - small DVE/ACT finishing pass computes std, DMA out per block.
"""

import numpy as np

N_NODES = 100000
N_FEAT = 64
N_EDGES = 1600000
P = 128
NCORES = 8
NB = 98                 # blocks per core
NBLK = NCORES * NB      # 784
GB = 7                  # blocks per group; 98 = 14*7
NQUART = 4
NQ = N_NODES // NQUART  # rows per src quarter (25000 < 32768 for int16 idx)
EPS = 1e-8
MM_DT = "bfloat16"      # "float32" | "bfloat16" for matmul operands

_CACHE = {}


def _build_program(n_nodes, f, nb, tq, gb, nq, mm_dt):
    import concourse.bass as bass
    import concourse.bacc as bacc
    import concourse.mybir as mybir
    import concourse.tile as tile

    F32 = mybir.dt.float32
    I16 = mybir.dt.int16
    MDT = getattr(mybir.dt, mm_dt)
    AO = mybir.AluOpType
    AF = mybir.ActivationFunctionType

    t = NQUART * tq            # tiles (columns) per block
    W = 2 * f + 1              # 129
    C = nb * t                 # total columns per core
    gcols = gb * t             # columns per group
    qcols = gb * tq            # columns per (group, quarter)
    ng = nb // gb
    nidx = qcols * P           # indices per gather
    i16c = nidx // 16          # idx16 cols per gather

    nc = bacc.Bacc(num_swdge_queues=4)
    xd = nc.declare_dram_parameter("x", [n_nodes, f], F32, isOutput=False)
    gidxd = nc.declare_dram_parameter(
        "gidx", [P, ng * NQUART * i16c], I16, isOutput=False)
    tgtd = nc.declare_dram_parameter("tgt", [P, C], F32, isOutput=False)
    outd = nc.declare_dram_parameter("out", [nb * P, f], F32, isOutput=True)

    with tile.TileContext(nc) as tc:
        with (
            tc.tile_pool(name="const", bufs=1) as constp,
            tc.tile_pool(name="io", bufs=2) as iop,
            tc.tile_pool(name="msg", bufs=2) as msgp,
            tc.tile_pool(name="oh", bufs=6) as ohp,
            tc.tile_pool(name="fin", bufs=4) as finp,
            tc.tile_pool(name="ov", bufs=4) as ovp,
            tc.tile_pool(name="ps", bufs=8, space="PSUM") as psump,
        ):
            # 4-wide iota [128, 4*128]: value = column index % 128
            iota4 = constp.tile([P, 4 * P], F32)
            nc.gpsimd.iota(iota4[:], pattern=[[0, 4], [1, P]], base=0,
                           channel_multiplier=0,
                           allow_small_or_imprecise_dtypes=True)

            for g in range(ng):
                idx = iop.tile([P, NQUART * i16c], I16, tag="idx")
                tg = iop.tile([P, gcols], F32, tag="tg")
                nc.sync.dma_start(
                    out=idx[:],
                    in_=gidxd[:, g * NQUART * i16c:(g + 1) * NQUART * i16c])
                nc.sync.dma_start(
                    out=tg[:], in_=tgtd[:, g * gcols:(g + 1) * gcols])
                tgv = iop.tile([P, gcols], F32, tag="tgv")
                nc.vector.tensor_copy(out=tgv[:], in_=tg[:])

                gbuf = msgp.tile([P, gcols * f], F32, tag="g")
                g3 = gbuf[:].rearrange("p (c e) -> p c e", e=f)
                for qq in range(NQUART):
                    nc.gpsimd.dma_gather(
                        out_ap=g3[:, qq * qcols:(qq + 1) * qcols, :],
                        in_ap=xd[qq * nq:(qq + 1) * nq, :],
                        idxs_ap=idx[:, qq * i16c:(qq + 1) * i16c],
                        num_idxs=nidx,
                        num_idxs_reg=nidx,
                        elem_size=f,
                        single_packet=False,
                        queue_num=qq,
                    )
                sqx = msgp.tile([P, gcols * W], MDT, tag="sqx")
                s3 = sqx[:].rearrange("p (c w) -> p c w", w=W)
                nc.scalar.activation(out=s3[:, :, 0:f], in_=g3[:, :, :],
                                     func=AF.Copy)
                nc.scalar.square(out=s3[:, :, f:2 * f], in_=g3[:, :, :])
                nc.scalar.activation(out=s3[:, :, 2 * f:W], in_=g3[:, :, 0:1],
                                     func=AF.Copy, bias=1.0, scale=0.0)

                pss = [psump.tile([P, W], F32, tag="ps", name=f"ps_{g}_{bl}")
                       for bl in range(gb)]
                for pk in range(gcols // 4):
                    oh4 = ohp.tile([P, 4 * P], MDT)
                    nc.vector.tensor_tensor(
                        out=oh4[:].rearrange("p (c e) -> p c e", e=P),
                        in0=tgv[:, 4 * pk:4 * pk + 4]
                            .rearrange("p (c u) -> p c u", u=1)
                            .to_broadcast([P, 4, P]),
                        in1=iota4[:].rearrange("p (c e) -> p c e", e=P),
                        op=AO.is_equal,
                    )
                    for i in range(4):
                        cl = 4 * pk + i
                        qq = cl // qcols
                        r = cl % qcols
                        bl = r // tq
                        j = r % tq
                        nc.tensor.matmul(
                            out=pss[bl][:],
                            lhsT=oh4[:, i * P:(i + 1) * P],
                            rhs=sqx[:, cl * W:(cl + 1) * W],
                            start=(qq == 0 and j == 0),
                            stop=(qq == NQUART - 1 and j == tq - 1),
                        )
                for bl in range(gb):
                    b = g * gb + bl
                    ps = pss[bl]
                    cnt = finp.tile([P, 1], F32, tag="cnt")
                    nc.vector.tensor_scalar(
                        out=cnt[:], in0=ps[:, 2 * f:W],
                        scalar1=float(EPS), scalar2=None, op0=AO.max)
                    rec = finp.tile([P, 1], F32, tag="rec")
                    nc.vector.reciprocal(out=rec[:], in_=cnt[:])
                    mean = finp.tile([P, f], F32, tag="mean")
                    nc.vector.tensor_scalar_mul(
                        out=mean[:], in0=ps[:, 0:f], scalar1=rec[:])
                    ex2 = finp.tile([P, f], F32, tag="ex2")
                    nc.vector.tensor_scalar_mul(
                        out=ex2[:], in0=ps[:, f:2 * f], scalar1=rec[:])
                    var = finp.tile([P, f], F32, tag="var")
                    nc.vector.tensor_tensor(
                        out=var[:], in0=mean[:], in1=mean[:], op=AO.mult)
                    nc.vector.tensor_tensor(
                        out=var[:], in0=ex2[:], in1=var[:], op=AO.subtract)
                    nc.vector.tensor_scalar(
                        out=var[:], in0=var[:], scalar1=0.0, scalar2=None,
                        op0=AO.max)
                    std = ovp.tile([P, f], F32, tag="std")
                    nc.scalar.sqrt(out=std[:], in_=var[:])
                    mask = finp.tile([P, 1], F32, tag="mask")
                    nc.vector.tensor_scalar(
                        out=mask[:], in0=ps[:, 2 * f:W],
                        scalar1=1.5, scalar2=None, op0=AO.is_gt)
                    nc.vector.tensor_scalar_mul(
                        out=std[:], in0=std[:], scalar1=mask[:])
                    nc.sync.dma_start(
                        out=outd[b * P:(b + 1) * P, :], in_=std[:])
    return nc


def _host_prep(x, edge_index):
    src = np.asarray(edge_index[0], dtype=np.int64)
    tgt = np.asarray(edge_index[1], dtype=np.int64)
    n_edges = src.shape[0]
    counts = np.bincount(tgt, minlength=N_NODES)

    # serpentine deal of count-sorted nodes into NBLK blocks of <=128 slots
    order = np.argsort(-counts, kind="stable")
    ranks = np.arange(N_NODES)
    rounds = ranks // NBLK
    pos = ranks % NBLK
    blk_of_rank = np.where(rounds % 2 == 0, pos, NBLK - 1 - pos)
    blk = np.empty(N_NODES, np.int64)
    slot = np.empty(N_NODES, np.int64)
    blk[order] = blk_of_rank
    slot[order] = rounds
    assert slot.max() < P

    eb = blk[tgt]                      # edge -> block
    eq = src // NQ                     # edge -> src quarter
    es = slot[tgt]                     # edge -> slot in block
    seg = eb * NQUART + eq             # edge -> (block, quarter) segment
    segsums = np.bincount(seg, minlength=NBLK * NQUART)
    tq = int(np.ceil(segsums.max() / P))
    cap = tq * P

    order_e = np.argsort(seg, kind="stable")
    segs = seg[order_e]
    starts = np.zeros(NBLK * NQUART, np.int64)
    np.cumsum(segsums[:-1], out=starts[1:])
    within = np.arange(n_edges) - starts[segs]
    flat = segs * cap + within

    gidxq = np.zeros((NBLK, NQUART, cap), np.int16)
    tgtq = np.full((NBLK, NQUART, cap), -1.0, np.float32)
    gidxq.reshape(-1)[flat] = (src[order_e] % NQ).astype(np.int16)
    tgtq.reshape(-1)[flat] = es[order_e].astype(np.float32)

    xf = np.ascontiguousarray(np.asarray(x, dtype=np.float32))
    ng = NB // GB
    i16c = GB * cap // 16

    in_maps = []
    for c in range(NCORES):
        tb = tgtq[c * NB:(c + 1) * NB]          # [NB, 4, cap]
        gi = gidxq[c * NB:(c + 1) * NB]
        # tgt columns: (group, quarter, block, tile) -> [P, C]
        tcore = (tb.reshape(ng, GB, NQUART, cap)
                 .transpose(0, 2, 1, 3)          # [ng, 4, GB, cap]
                 .reshape(ng * NQUART * GB * tq, P).T)
        # idx16: per (group, quarter): stream of GB*cap idxs wrapped %16
        gs = (gi.reshape(ng, GB, NQUART, cap)
              .transpose(0, 2, 1, 3)             # [ng, 4, GB, cap]
              .reshape(ng * NQUART, GB * cap))   # per-gather streams
        idx16 = np.ascontiguousarray(
            np.tile(gs.reshape(ng * NQUART, i16c, 16).transpose(0, 2, 1)
                    .reshape(ng * NQUART * 16, i16c)
                    .reshape(ng * NQUART, 16, i16c)
                    .transpose(1, 0, 2).reshape(16, ng * NQUART * i16c),
                    (8, 1)))
        in_maps.append({
            "x": xf,
            "gidx": idx16,
            "tgt": np.ascontiguousarray(tcore),
        })
    return tq, in_maps, blk, slot


def _run(x, edge_index, trace=False):
    from concourse.bass_utils import run_bass_kernel_spmd

    tq, in_maps, blk, slot = _host_prep(x, edge_index)
    key = ("prog", tq, MM_DT)
    if key not in _CACHE:
        nc_ = _build_program(N_NODES, N_FEAT, NB, tq, GB, NQ, MM_DT)
        nc_.finalize()
        _CACHE[key] = nc_
    nc = _CACHE[key]
    res = run_bass_kernel_spmd(
        nc, in_maps, core_ids=list(range(NCORES)), trace=trace)

    outs = [np.asarray(r["out"]) for r in res.results]
    out_full = np.empty((N_NODES, N_FEAT), np.float32)
    cores = blk // NB
    rows = (blk % NB) * P + slot
    for c in range(NCORES):
        m = cores == c
        out_full[m] = outs[c][rows[m]]
    return out_full, res


def kernel(**inputs):
    out, _ = _run(inputs["x"], inputs["edge_index"], trace=False)
    return out



# revision 4
# speedup vs baseline: 2.8105x; 1.2046x over previous
"""GNN message-passing (std aggregator) on 8 TRN2 NeuronCores.

Math per target node: count, S1 = sum x[src], S2 = sum x[src]^2;
mean = S1/max(count,eps); var = S2/count - mean^2;
std = sqrt(max(var,0)), zeroed where count <= 1.

Strategy: shard TARGET nodes across cores (no collectives). Host packs nodes
into 128-bin blocks balanced by in-degree (serpentine deal), buckets edges by
(block, src-quarter) with uniform tile capacity tq per (block,quarter) so one
NEFF serves all cores. Host also pre-packs a [N,128] bf16 table
xpack = [x | x^2] (256B rows) and per-node rz = (count>1)/max(count,eps).
Device per core, per group of GB blocks:
  - 4x dma_gather on swdge queues 0-3 (parallel Q7 core pairs) pulls
    xpack rows straight into matmul-ready rhs tiles [e,128] bf16,
  - DVE builds 4-wide one-hot tiles (slot-vs-iota is_equal),
  - PE matmul-accumulates [128 bins x 128] = [S1 | S2] in PSUM,
  - batched group finishing: me = psum*rz (fused PSUM read), var =
    ex2 - mean^2 clamped, ACT sqrt, one DMA out per group.
"""

import numpy as np
import ml_dtypes

N_NODES = 100000
N_FEAT = 64
N_EDGES = 1600000
P = 128
NCORES = 8
NB = 98                 # blocks per core
NBLK = NCORES * NB      # 784
GB = 7                  # blocks per group; 98 = 14*7
NQUART = 4
NQ = N_NODES // NQUART  # rows per src quarter (25000 < 32768 for int16 idx)
EPS = 1e-8
BF16 = ml_dtypes.bfloat16

_CACHE = {}


def _build_program(n_nodes, f, nb, tq, gb, nq):
    import concourse.bass as bass
    import concourse.bacc as bacc
    import concourse.mybir as mybir
    import concourse.tile as tile

    F32 = mybir.dt.float32
    I16 = mybir.dt.int16
    BF = mybir.dt.bfloat16
    AO = mybir.AluOpType
    AF = mybir.ActivationFunctionType

    w = 2 * f                  # 128 = [x | x^2]
    t = NQUART * tq            # tiles (columns of 128 edges) per block
    C = nb * t                 # total 128-edge packets per core
    gcols = gb * t             # packets per group
    qcols = gb * tq            # packets per (group, quarter)
    ng = nb // gb
    nidx = qcols * P           # indices per gather
    i16c = nidx // 16          # idx16 cols per gather

    nc = bacc.Bacc(num_swdge_queues=4)
    xd = nc.declare_dram_parameter("xpack", [n_nodes, w], BF, isOutput=False)
    gidxd = nc.declare_dram_parameter(
        "gidx", [P, ng * NQUART * i16c], I16, isOutput=False)
    tgtd = nc.declare_dram_parameter("tgt", [P, C], F32, isOutput=False)
    rzd = nc.declare_dram_parameter("rz", [P, nb], F32, isOutput=False)
    outd = nc.declare_dram_parameter("out", [ng * P, gb * f], F32,
                                     isOutput=True)

    with tile.TileContext(nc) as tc:
        with (
            tc.tile_pool(name="const", bufs=1) as constp,
            tc.tile_pool(name="io", bufs=2) as iop,
            tc.tile_pool(name="msg", bufs=2) as msgp,
            tc.tile_pool(name="oh", bufs=6) as ohp,
            tc.tile_pool(name="fin", bufs=2) as finp,
            tc.tile_pool(name="ov", bufs=2) as ovp,
            tc.tile_pool(name="ps", bufs=8, space="PSUM") as psump,
        ):
            # 4-wide iota [128, 4*128]: value = column index % 128
            iota4 = constp.tile([P, 4 * P], F32)
            nc.gpsimd.iota(iota4[:], pattern=[[0, 4], [1, P]], base=0,
                           channel_multiplier=0,
                           allow_small_or_imprecise_dtypes=True)
            rz = constp.tile([P, nb], F32)
            nc.sync.dma_start(out=rz[:], in_=rzd[:, :])

            for g in range(ng):
                idx = iop.tile([P, NQUART * i16c], I16, tag="idx")
                tg = iop.tile([P, gcols], F32, tag="tg")
                nc.sync.dma_start(
                    out=idx[:],
                    in_=gidxd[:, g * NQUART * i16c:(g + 1) * NQUART * i16c])
                nc.sync.dma_start(
                    out=tg[:], in_=tgtd[:, g * gcols:(g + 1) * gcols])

                sqx = msgp.tile([P, gcols * w], BF, tag="sqx")
                s3 = sqx[:].rearrange("p (c e) -> p c e", e=w)
                for qq in range(NQUART):
                    nc.gpsimd.dma_gather(
                        out_ap=s3[:, qq * qcols:(qq + 1) * qcols, :],
                        in_ap=xd[qq * nq:(qq + 1) * nq, :],
                        idxs_ap=idx[:, qq * i16c:(qq + 1) * i16c],
                        num_idxs=nidx,
                        num_idxs_reg=nidx,
                        elem_size=w,
                        single_packet=False,
                        queue_num=qq,
                    )

                pss = [psump.tile([P, w], F32, tag="ps", name=f"ps_{g}_{bl}")
                       for bl in range(gb)]
                for pk in range(gcols // 4):
                    oh4 = ohp.tile([P, 4 * P], BF)
                    nc.vector.tensor_tensor(
                        out=oh4[:].rearrange("p (c e) -> p c e", e=P),
                        in0=tg[:, 4 * pk:4 * pk + 4]
                            .rearrange("p (c u) -> p c u", u=1)
                            .to_broadcast([P, 4, P]),
                        in1=iota4[:].rearrange("p (c e) -> p c e", e=P),
                        op=AO.is_equal,
                    )
                    for i in range(4):
                        cl = 4 * pk + i
                        qq = cl // qcols
                        r = cl % qcols
                        bl = r // tq
                        j = r % tq
                        nc.tensor.matmul(
                            out=pss[bl][:],
                            lhsT=oh4[:, i * P:(i + 1) * P],
                            rhs=sqx[:, cl * w:(cl + 1) * w],
                            start=(qq == 0 and j == 0),
                            stop=(qq == NQUART - 1 and j == tq - 1),
                        )

                # group finishing pass: me = psum * rz (fused PSUM read,
                # per block), then batched var/std over [P, gb*f]
                me = finp.tile([P, gb * w], F32, tag="me")
                m3 = me[:].rearrange("p (b e) -> p b e", e=w)
                for bl in range(gb):
                    b = g * gb + bl
                    nc.vector.tensor_tensor(
                        out=m3[:, bl:bl + 1, :],
                        in0=pss[bl][:].rearrange("p (u e) -> p u e", u=1),
                        in1=rz[:, b:b + 1]
                            .rearrange("p (u e) -> p u e", u=1)
                            .to_broadcast([P, 1, w]),
                        op=AO.mult,
                    )
                var = finp.tile([P, gb * f], F32, tag="var")
                v3 = var[:].rearrange("p (b e) -> p b e", e=f)
                nc.vector.tensor_tensor(
                    out=v3[:, :, :], in0=m3[:, :, 0:f], in1=m3[:, :, 0:f],
                    op=AO.mult)
                nc.vector.tensor_tensor(
                    out=v3[:, :, :], in0=m3[:, :, f:w], in1=v3[:, :, :],
                    op=AO.subtract)
                nc.vector.tensor_scalar(
                    out=var[:], in0=var[:], scalar1=0.0, scalar2=None,
                    op0=AO.max)
                std = ovp.tile([P, gb * f], F32, tag="std")
                nc.scalar.sqrt(out=std[:], in_=var[:])
                nc.sync.dma_start(
                    out=outd[g * P:(g + 1) * P, :], in_=std[:])
    return nc


def _host_prep(x, edge_index):
    src = np.asarray(edge_index[0], dtype=np.int64)
    tgt = np.asarray(edge_index[1], dtype=np.int64)
    n_edges = src.shape[0]
    counts = np.bincount(tgt, minlength=N_NODES)

    # serpentine deal of count-sorted nodes into NBLK blocks of <=128 slots
    order = np.argsort(-counts, kind="stable")
    ranks = np.arange(N_NODES)
    rounds = ranks // NBLK
    pos = ranks % NBLK
    blk_of_rank = np.where(rounds % 2 == 0, pos, NBLK - 1 - pos)
    blk = np.empty(N_NODES, np.int64)
    slot = np.empty(N_NODES, np.int64)
    blk[order] = blk_of_rank
    slot[order] = rounds
    assert slot.max() < P

    eb = blk[tgt]                      # edge -> block
    eq = src // NQ                     # edge -> src quarter
    es = slot[tgt]                     # edge -> slot in block
    seg = eb * NQUART + eq             # edge -> (block, quarter) segment
    segsums = np.bincount(seg, minlength=NBLK * NQUART)
    tq = int(np.ceil(segsums.max() / P))
    cap = tq * P

    # within each segment, order edges by src row for DRAM gather locality
    order_e = np.lexsort((src, seg))
    segs = seg[order_e]
    starts = np.zeros(NBLK * NQUART, np.int64)
    np.cumsum(segsums[:-1], out=starts[1:])
    within = np.arange(n_edges) - starts[segs]
    flat = segs * cap + within

    gidxq = np.zeros((NBLK, NQUART, cap), np.int16)
    tgtq = np.full((NBLK, NQUART, cap), -1.0, np.float32)
    gidxq.reshape(-1)[flat] = (src[order_e] % NQ).astype(np.int16)
    tgtq.reshape(-1)[flat] = es[order_e].astype(np.float32)

    # packed per-node table [x | x^2] in bf16 (256B rows)
    xf = np.asarray(x, dtype=np.float32)
    xpack = np.empty((N_NODES, 2 * N_FEAT), BF16)
    xpack[:, :N_FEAT] = xf.astype(BF16)
    xpack[:, N_FEAT:] = (xf * xf).astype(BF16)
    xpack = np.ascontiguousarray(xpack)

    # per-node (count>1)/max(count,eps), laid out [slot, block] per core
    rz_node = np.where(counts > 1, 1.0 / np.maximum(counts, EPS), 0.0)
    rz_node = rz_node.astype(np.float32)
    rz_all = np.zeros((NBLK, P), np.float32)
    rz_all[blk, slot] = rz_node
    rz_all = rz_all.reshape(NCORES, NB, P)

    ng = NB // GB
    i16c = GB * cap // 16

    in_maps = []
    for c in range(NCORES):
        tb = tgtq[c * NB:(c + 1) * NB]          # [NB, 4, cap]
        gi = gidxq[c * NB:(c + 1) * NB]
        # tgt columns: (group, quarter, block, tile) -> [P, C]
        tcore = (tb.reshape(ng, GB, NQUART, cap)
                 .transpose(0, 2, 1, 3)          # [ng, 4, GB, cap]
                 .reshape(ng * NQUART * GB * tq, P).T)
        # idx16: per (group, quarter): stream of GB*cap idxs wrapped %16
        gs = (gi.reshape(ng, GB, NQUART, cap)
              .transpose(0, 2, 1, 3)             # [ng, 4, GB, cap]
              .reshape(ng * NQUART, GB * cap))   # per-gather streams
        idx16 = np.ascontiguousarray(
            np.tile(gs.reshape(ng * NQUART, i16c, 16).transpose(0, 2, 1)
                    .reshape(ng * NQUART * 16, i16c)
                    .reshape(ng * NQUART, 16, i16c)
                    .transpose(1, 0, 2).reshape(16, ng * NQUART * i16c),
                    (8, 1)))
        in_maps.append({
            "xpack": xpack,
            "gidx": idx16,
            "tgt": np.ascontiguousarray(tcore),
            "rz": np.ascontiguousarray(rz_all[c].T),   # [P, NB]
        })
    return tq, in_maps, blk, slot


def _run(x, edge_index, trace=False):
    from concourse.bass_utils import run_bass_kernel_spmd

    tq, in_maps, blk, slot = _host_prep(x, edge_index)
    key = ("prog", tq)
    if key not in _CACHE:
        nc_ = _build_program(N_NODES, N_FEAT, NB, tq, GB, NQ)
        nc_.finalize()
        _CACHE[key] = nc_
    nc = _CACHE[key]
    res = run_bass_kernel_spmd(
        nc, in_maps, core_ids=list(range(NCORES)), trace=trace)

    # out layout: [ng*P, GB*f]; block b = g*GB + bl lives at rows g*P + slot,
    # cols bl*f:(bl+1)*f
    out_full = np.empty((N_NODES, N_FEAT), np.float32)
    ng = NB // GB
    cores = blk // NB
    for c in range(NCORES):
        o = np.asarray(res.results[c]["out"]).reshape(ng, P, GB, N_FEAT)
        m = cores == c
        bc = blk[m] % NB
        out_full[m] = o[bc // GB, slot[m], bc % GB]
    return out_full, res


def kernel(**inputs):
    out, _ = _run(inputs["x"], inputs["edge_index"], trace=False)
    return out
